# revision 40
# baseline (speedup 1.0000x reference)
"""Multi-head attention block (B=8, S=1024, D=768, H=12) on 8 TRN2 NeuronCores.

Data-parallel: one batch element per core (attention is independent per batch).
Per-core pipeline (bf16 matmuls, fp32 PSUM accumulation):

  xT [D,S] uploaded pre-transposed from host (bf16), single DMA
  QT = Wq^T xT (+bq), KT = Wk^T xT (+bk)      [D,S]  (head pairs per 128-chunk)
  V  = x Wv (+bv)                             [S,D]  stored as [128,12,65] with
                                                      a ones column per head
  per head pair (2c,2c+1):
      S^T pair = K Q^T row-packed on PE array halves (two [128,1024] PSUM tiles)
      exp(scale*S^T) -> PT pair [Sk, Sq] bf16 per head
      per head: O'^T = [V_h,1]^T PT (accum Sk) -> rows 0:64 = O^T, row 64 =
                rowsum; OT_h = O'^T[0:64] * (1 / bcast(rowsum))
  out = O_cat @ Wp (+bp)                      [S,D]  (bf16 output, host-cast f32)

Schedule: the ACT engine is the steady-state pacer (~18.4us/pair of exp).
Each pair iteration interleaves at kc granularity: scores(c+1) groups are
round-robined with PV(c) partial accumulations and split qk(c+2) halves so
ACT receives a fresh score tile every ~2.3us and never starves at pair
boundaries.  PSUM: st2 (2x2 banks) rotates score tiles (also proj f_ps);
p2 (2x2 banks) holds ov pairs / qk / v transients.

Startup: single-shot DMAs split across both HWDGE queues (sync: xT, bq, bk,
Wq/Wk col-blocks 1:6; scalar: bv, bp, Wq/Wk col-block 0, Wv, Wp) so QT/KT
chunk 0 starts as soon as xT lands; dummy PE warmup matmuls flip the HAM
clock-gate to 8/8 before real compute arrives.
"""

import numpy as np

B, S, DIM, H = 8, 1024, 768, 12
HD = DIM // H          # 64
SCALE = HD ** -0.5
N_CORES = 8
KC = DIM // 128        # 6 d-chunks
SC = S // 128          # 8 seq-chunks
ENABLE_PINS = False

_CACHE = {}


def _build():
    import concourse.mybir as mybir
    import concourse.tile as tile
    from concourse import bacc

    f32 = mybir.dt.float32
    bf16 = mybir.dt.bfloat16
    EXP = mybir.ActivationFunctionType.Exp

    nc = bacc.Bacc()

    xT_ext = nc.declare_dram_parameter("xT", [DIM, S], bf16, isOutput=False)
    Wq_ext = nc.declare_dram_parameter("Wq", [DIM, DIM], bf16, isOutput=False)
    bq_ext = nc.declare_dram_parameter("bq", [DIM], f32, isOutput=False)
    Wk_ext = nc.declare_dram_parameter("Wk", [DIM, DIM], bf16, isOutput=False)
    bk_ext = nc.declare_dram_parameter("bk", [DIM], f32, isOutput=False)
    Wv_ext = nc.declare_dram_parameter("Wv", [DIM, DIM], bf16, isOutput=False)
    bv_ext = nc.declare_dram_parameter("bv", [DIM], bf16, isOutput=False)
    Wp_ext = nc.declare_dram_parameter("Wp", [DIM, DIM], bf16, isOutput=False)
    bp_ext = nc.declare_dram_parameter("bp", [DIM], bf16, isOutput=False)
    out_ext = nc.declare_dram_parameter("out", [S, DIM], bf16, isOutput=True)

    HALVES = ((0, 512), (512, 1024))
    VHALVES = ((0, 512), (512, DIM))

    with tile.TileContext(nc) as tc:
        with tc.tile_pool(name="persist", bufs=1) as sb, \
             tc.tile_pool(name="ps", bufs=1, space="PSUM") as ps:

            # p2: single-slot transient accumulator (2 banks) — all users are
            # strictly sequential (warmup, bias, qk, v, one ov at a time).
            # st2: 3-slot rotation (6 banks) for score tiles — the extra slot
            # gives the ACT exp stream a deeper backlog so scheduler slop no
            # longer starves it; norm's bc_ps borrows a slot transiently.
            def p2(name, shape=(128, S), dtype=f32):
                return ps.tile(list(shape), dtype, tag="p2", bufs=1, name=name)

            def st2(name):
                return ps.tile([128, S], f32, tag="st2", bufs=3, name=name)

            # ---- constants ----
            ones2d = sb.tile([128, 128], bf16)
            nc.vector.memset(ones2d, 1.0)
            rs_z = sb.tile([128, DIM], bf16)
            nc.vector.memset(rs_z, 0.0)
            rs_zs = sb.tile([128, S], bf16)
            nc.vector.memset(rs_zs, 0.0)

            # PE warmup: dummy matmuls keep the array busy from the end of the
            # framework preamble so the HAM clock-gate flips to 8/8 (~3.4us)
            # and stays there until the input DMAs land.
            warm_ps = p2("warm", (128, 512))
            for w in range(145):
                nc.tensor.matmul(
                    warm_ps[:, (w % 4) * 128:(w % 4) * 128 + 128],
                    ones2d, ones2d, start=True, stop=True)

            # ---- input DMAs: single-shot per tensor, split across queues.
            # Critical path first (x, Wq, Wk); Wv/Wp issues are deferred via
            # a dep on the first score matmul so their transfers do not steal
            # DMA bandwidth from the QT0/KT0 inputs.
            # x in two half-DMAs: QT0's k=0..2 accumulation starts as soon as
            # the first half lands, hiding half the transfer behind compute.
            xsb = sb.tile([128, KC, S], bf16, name="xsb")
            nc.sync.dma_start(
                out=xsb[:, 0:3, :],
                in_=xT_ext[0:3 * 128, :].rearrange("(c p) s -> p c s", p=128))
            nc.sync.dma_start(
                out=xsb[:, 3:KC, :],
                in_=xT_ext[3 * 128:DIM, :].rearrange("(c p) s -> p c s", p=128))
            Wq_sb = sb.tile([128, KC, DIM], bf16, name="Wq_sb")
            Wk_sb = sb.tile([128, KC, DIM], bf16, name="Wk_sb")
            # col-block 0 (needed by QT0/KT0) goes on the scalar queue so it
            # arrives with xT; the rest follows on sync.
            nc.sync.dma_start(
                out=Wq_sb[:, :, 128:DIM],
                in_=Wq_ext[:, 128:DIM].rearrange("(c p) n -> p c n", p=128))
            nc.sync.dma_start(
                out=Wk_sb[:, :, 128:DIM],
                in_=Wk_ext[:, 128:DIM].rearrange("(c p) n -> p c n", p=128))
            bq_sb = sb.tile([128, KC], f32)
            nc.sync.dma_start(out=bq_sb, in_=bq_ext[:].rearrange("(c p) -> p c", p=128))
            bk_sb = sb.tile([128, KC], f32)
            nc.sync.dma_start(out=bk_sb, in_=bk_ext[:].rearrange("(c p) -> p c", p=128))

            nc.scalar.dma_start(
                out=Wq_sb[:, :, 0:128],
                in_=Wq_ext[:, 0:128].rearrange("(c p) n -> p c n", p=128))
            nc.scalar.dma_start(
                out=Wk_sb[:, :, 0:128],
                in_=Wk_ext[:, 0:128].rearrange("(c p) n -> p c n", p=128))
            bv_row = sb.tile([1, DIM], bf16)
            nc.scalar.dma_start(out=bv_row, in_=bv_ext[:].rearrange("(a d) -> a d", a=1))
            bp_row = sb.tile([1, DIM], bf16)
            nc.scalar.dma_start(out=bp_row, in_=bp_ext[:].rearrange("(a d) -> a d", a=1))
            Wv_sb = sb.tile([128, KC, DIM], bf16, name="Wv_sb")
            wv_dma = nc.scalar.dma_start(
                out=Wv_sb, in_=Wv_ext[:].rearrange("(c p) n -> p c n", p=128))
            Wp_sb = sb.tile([128, KC, DIM], bf16, name="Wp_sb")
            wp_dma = nc.scalar.dma_start(
                out=Wp_sb, in_=Wp_ext[:].rearrange("(c p) n -> p c n", p=128))

            xT = [xsb[:, c, :] for c in range(KC)]

            # broadcast bv/bp across 128 partitions: full-util K=128 matmul
            # against the zero-padded carrier (row 0 = bias, rows 1:128 = 0).
            bv_bc = sb.tile([128, DIM], f32)
            bp_bc = sb.tile([128, DIM], f32)
            for row, bc in ((bv_row, bv_bc), (bp_row, bp_bc)):
                nc.vector.tensor_copy(rs_z[0:1, 0:DIM], row[0:1, :])
                bc_ps = p2("bias_ps")
                for n0, n1 in VHALVES:
                    nc.tensor.matmul(bc_ps[:, n0:n1], ones2d,
                                     rs_z[:, n0:n1], start=True, stop=True)
                nc.scalar.copy(bc, bc_ps[:, 0:DIM])

            # persistent activation storage; V12 col 64 is the ones column
            # that makes PV also emit the softmax rowsum (out row 64).
            QT = [sb.tile([128, S], bf16, name=f"QT{c}") for c in range(KC)]
            KT = [sb.tile([128, S], bf16, name=f"KT{c}") for c in range(KC)]
            V12 = [sb.tile([128, H, HD + 1], bf16, name=f"V12_{s8}") for s8 in range(SC)]
            for s8 in range(SC):
                nc.vector.memset(V12[s8][:, :, HD:HD + 1], 1.0)

            # Ordering pin: anchor filler blocks behind the most recent score
            # group so the greedy scheduler cannot hoist them ahead of it and
            # starve the ACT engine (the steady-state pacer).
            anchor = [None]

            def pin(inst):
                if ENABLE_PINS and anchor[0] is not None:
                    tile.add_dep_helper(inst.ins, anchor[0].ins, sync=False,
                                        reason="keep ACT fed")

            # QT / KT chunk m, split in two emission halves (k 0:3 / 3:6) so
            # the pair loop can interleave them between score groups.
            def qk_half(W_sb, bias_sb, dst, m, q_ps, first, pinned=False):
                ks = range(0, 3) if first else range(3, KC)
                for ki, k in enumerate(ks):
                    for ni, (n0, n1) in enumerate(HALVES):
                        mm = nc.tensor.matmul(
                            q_ps[:, n0:n1],
                            W_sb[:, k, m * 128:(m + 1) * 128],
                            xT[k][:, n0:n1],
                            start=(k == 0), stop=(k == KC - 1))
                        if pinned and ki == 0 and ni == 0:
                            pin(mm)
                if not first:
                    nc.vector.tensor_scalar_add(dst[m], q_ps, bias_sb[:, m:m + 1])

            def qk_chunk(W_sb, bias_sb, dst, m):
                q_ps = p2("q_ps")
                qk_half(W_sb, bias_sb, dst, m, q_ps, True)
                qk_half(W_sb, bias_sb, dst, m, q_ps, False)

            # V natural layout: out[seq, d] = x @ Wv; +bv broadcast; ->bf16.
            def emit_v_chunk(s8):
                v_ps = p2("v_ps")
                for k in range(KC):
                    for n0, n1 in VHALVES:
                        nc.tensor.matmul(
                            v_ps[:, n0:n1],
                            xT[k][:, s8 * 128:(s8 + 1) * 128],
                            Wv_sb[:, k, n0:n1],
                            start=(k == 0), stop=(k == KC - 1))
                nc.vector.tensor_add(
                    V12[s8][:, :, 0:HD],
                    v_ps[:, 0:DIM].rearrange("p (h d) -> p h d", h=H),
                    bv_bc[:].rearrange("p (h d) -> p h d", h=H))

            qk_chunk(Wq_sb, bq_sb, QT, 0)
            qk_chunk(Wk_sb, bk_sb, KT, 0)

            # ---- attention + output projection ----
            with tc.tile_pool(name="pb", bufs=1) as pb:
                OT = [pb.tile([128, S], bf16, name=f"OT{c}") for c in range(KC)]

                # Schraudolph fast-exp on DVE for the offloaded k-chunks:
                # exp(SCALE*s) = bitcast_f32(int32(EA*s + EB)).  Rebalances
                # the exp stream (the pacer) from ACT onto the idle DVE at
                # ~1.7% rms approximation error on those chunks.
                # Single DVE op: t = (EA*s + EB) in fp32, convert to int16
                # (round) and land the bits directly in the bf16 pt tile --
                # int16 steps are exactly bf16 ulps of 2^(scale*s/ln2).
                EA = float((1 << 23) * SCALE / np.log(2.0) / 65536.0)
                EB = float((127 * (1 << 23) - 366393) / 65536.0)
                OFF_KCS = ()
                i16 = mybir.dt.int16
                MULT, ADD = mybir.AluOpType.mult, mybir.AluOpType.add

                def dve_exp(dst, st):
                    nc.vector.tensor_scalar(
                        out=dst.bitcast(i16), in0=st, scalar1=EA, scalar2=EB,
                        op0=MULT, op1=ADD)

                def sc_group(c, kc, pt):
                    # scores + exp for (pair c, k-chunk kc): row-packed
                    # matmuls on array halves run concurrently.
                    st_e = st2("st_e")
                    st_o = st2("st_o")
                    for n0, n1 in HALVES:
                        nc.tensor.matmul(
                            st_e[:, n0:n1],
                            KT[c][0:HD, kc * 128:(kc + 1) * 128],
                            QT[c][0:HD, n0:n1],
                            start=True, stop=True)
                        mm = nc.tensor.matmul(
                            st_o[:, n0:n1],
                            KT[c][HD:128, kc * 128:(kc + 1) * 128],
                            QT[c][HD:128, n0:n1],
                            start=True, stop=True)
                    anchor[0] = mm
                    p_e = pb.tile([128, S], bf16, tag=f"pt{kc}e", bufs=2, name=f"pt{kc}e")
                    p_o = pb.tile([128, S], bf16, tag=f"pt{kc}o", bufs=2, name=f"pt{kc}o")
                    if kc in OFF_KCS:
                        dve_exp(p_e, st_e)
                        dve_exp(p_o, st_o)
                    else:
                        nc.scalar.activation(p_e, st_e, EXP, scale=SCALE)
                        nc.scalar.activation(p_o, st_o, EXP, scale=SCALE)
                    pt[0].append(p_e)
                    pt[1].append(p_o)

                def pv_part(c, half, ptl, ov, kcs, pinned=True):
                    for j, kc in enumerate(kcs):
                        for ni, (n0, n1) in enumerate(HALVES):
                            mm = nc.tensor.matmul(
                                ov[0:HD + 1, n0:n1],
                                V12[kc][:, 2 * c + half, :],
                                ptl[kc][:, n0:n1],
                                start=(kc == 0), stop=(kc == SC - 1))
                            if pinned and j == 0 and ni == 0:
                                pin(mm)

                def norm(c, half, ov):
                    # rowsum (row 64 of ov) -> zero-padded carrier ->
                    # broadcast via full-util K=128 matmul -> 1/x -> normalize
                    nc.vector.tensor_copy(rs_zs[0:1, :], ov[HD:HD + 1, :])
                    bc_ps = st2("bc_ps")
                    for ni, (n0, n1) in enumerate(HALVES):
                        mm = nc.tensor.matmul(bc_ps[:, n0:n1], ones2d,
                                              rs_zs[:, n0:n1], start=True, stop=True)
                        if ni == 0:
                            pin(mm)
                    rbc = pb.tile([HD, S], f32, tag="rbc", bufs=2, name="rbc")
                    nc.vector.reciprocal_approx_fast(rbc, bc_ps[0:HD, :])
                    base = half * HD
                    nc.vector.tensor_mul(OT[c][base:base + HD, :], ov[0:HD, :], rbc)

                # pair 0 fill: score groups first (the exp stream must start
                # flowing), then qk(1) (pair 1's scores gate on it), then the
                # V chunks — unpinned, so the scheduler drops them into the
                # ACT-paced PE stalls.
                pts = ([], [])
                for kc in range(SC):
                    sc_group(0, kc, pts)
                    if kc == 0:
                        # release the deferred Wv/Wp transfers once the
                        # critical-path inputs have landed
                        tile.add_dep_helper(wv_dma.ins, anchor[0].ins,
                                            reason="defer Wv transfer")
                        tile.add_dep_helper(wp_dma.ins, anchor[0].ins,
                                            reason="defer Wp transfer")
                qk_chunk(Wq_sb, bq_sb, QT, 1)
                qk_chunk(Wk_sb, bk_sb, KT, 1)
                for kc in range(SC):
                    emit_v_chunk(kc)

                # steady state: scores(c+1) round-robined with PV(c) partials
                # and qk(c+2) halves, ACT-paced.
                for c in range(KC):
                    last = c + 1 >= KC
                    ptn = ([], [])

                    def sc(kc):
                        if not last:
                            sc_group(c + 1, kc, ptn)

                    sc(0)
                    sc(1)
                    ov0 = p2("ov0")
                    # first filler of the iteration is unpinned: it is the
                    # boundary slack-filler while ACT drains the prior pair
                    pv_part(c, 0, pts[0], ov0, range(0, 4), pinned=False)
                    sc(2)
                    pv_part(c, 0, pts[0], ov0, range(4, SC))
                    sc(3)
                    norm(c, 0, ov0)
                    # single-slot p2: ov1 reuses ov0's slot once norm(c,0)'s
                    # multiply has drained
                    ov1 = p2("ov1")
                    pv_part(c, 1, pts[1], ov1, range(0, 4))
                    sc(4)
                    pv_part(c, 1, pts[1], ov1, range(4, SC))
                    sc(5)
                    norm(c, 1, ov1)
                    if c + 2 < KC:
                        q_psq = p2("q_ps")
                        qk_half(Wq_sb, bq_sb, QT, c + 2, q_psq, True, pinned=True)
                        sc(6)
                        qk_half(Wq_sb, bq_sb, QT, c + 2, q_psq, False, pinned=True)
                        sc(7)
                        q_psk = p2("q_ps")
                        qk_half(Wk_sb, bk_sb, KT, c + 2, q_psk, True, pinned=True)
                        qk_half(Wk_sb, bk_sb, KT, c + 2, q_psk, False)
                    else:
                        sc(6)
                        sc(7)
                    pts = ptn

                # out = O_cat @ Wp + bp  (bf16 store, host casts to f32).
                # Split each chunk's contraction at k=5 so the k=0..4 partial
                # (10 MMs) can overlap the last pair's PV/norm drain; only
                # the k=5 matmuls wait on the final heads.
                def proj_part(s8, f_ps, ks):
                    for k in ks:
                        for n0, n1 in VHALVES:
                            nc.tensor.matmul(
                                f_ps[:, n0:n1],
                                OT[k][:, s8 * 128:(s8 + 1) * 128],
                                Wp_sb[:, k, n0:n1],
                                start=(k == 0), stop=(k == KC - 1))

                def proj_fin(s8, f_ps):
                    proj_part(s8, f_ps, range(KC - 1, KC))
                    fin = pb.tile([128, DIM], bf16, tag="fin", bufs=2, name="fin")
                    nc.vector.tensor_add(fin, f_ps[:, 0:DIM], bp_bc)
                    eng = nc.sync if s8 % 2 == 0 else nc.scalar
                    eng.dma_start(out=out_ext[s8 * 128:(s8 + 1) * 128, :], in_=fin)

                for s8 in range(SC):
                    f_ps = st2("f_ps")
                    proj_part(s8, f_ps, range(KC - 1))
                    proj_fin(s8, f_ps)

    nc.compile()
    return nc


def get_nc():
    if "nc" not in _CACHE:
        _CACHE["nc"] = _build()
    return _CACHE["nc"]


def make_in_maps(x, Wq, bq, Wk, bk, Wv, bv, Wp, bp):
    import ml_dtypes
    bfl = ml_dtypes.bfloat16
    shared = {
        "Wq": np.ascontiguousarray(np.asarray(Wq, np.float32).astype(bfl)),
        "bq": np.ascontiguousarray(np.asarray(bq, np.float32)),
        "Wk": np.ascontiguousarray(np.asarray(Wk, np.float32).astype(bfl)),
        "bk": np.ascontiguousarray(np.asarray(bk, np.float32)),
        "Wv": np.ascontiguousarray(np.asarray(Wv, np.float32).astype(bfl)),
        "bv": np.ascontiguousarray(np.asarray(bv, np.float32).astype(bfl)),
        "Wp": np.ascontiguousarray(np.asarray(Wp, np.float32).astype(bfl)),
        "bp": np.ascontiguousarray(np.asarray(bp, np.float32).astype(bfl)),
    }
    xb = np.asarray(x, np.float32).astype(bfl)
    return [{"xT": np.ascontiguousarray(xb[b].T), **shared} for b in range(N_CORES)]


def kernel(x, Wq, bq, Wk, bk, Wv, bv, Wp, bp):
    from concourse.bass_utils import run_bass_kernel_spmd

    nc = get_nc()
    in_maps = make_in_maps(x, Wq, bq, Wk, bk, Wv, bv, Wp, bp)
    res = run_bass_kernel_spmd(nc, in_maps, core_ids=list(range(N_CORES)))
    return np.stack(
        [res.results[i]["out"].astype(np.float32) for i in range(N_CORES)], axis=0)


# revision 41
# speedup vs baseline: 1.0125x; 1.0125x over previous
"""Multi-head attention block (B=8, S=1024, D=768, H=12) on 8 TRN2 NeuronCores.

Data-parallel: one batch element per core (attention is independent per batch).
Per-core pipeline (bf16 matmuls, fp32 PSUM accumulation):

  xT [D,S] uploaded pre-transposed from host (bf16), single DMA
  QT = Wq^T xT (+bq), KT = Wk^T xT (+bk)      [D,S]  (head pairs per 128-chunk)
  V  = x Wv (+bv)                             [S,D]  stored as [128,12,65] with
                                                      a ones column per head
  per head pair (2c,2c+1):
      S^T pair = K Q^T row-packed on PE array halves (two [128,1024] PSUM tiles)
      exp(scale*S^T) -> PT pair [Sk, Sq] bf16 per head
      per head: O'^T = [V_h,1]^T PT (accum Sk) -> rows 0:64 = O^T, row 64 =
                rowsum; OT_h = O'^T[0:64] * (1 / bcast(rowsum))
  out = O_cat @ Wp (+bp)                      [S,D]  (bf16 output, host-cast f32)

Schedule: the ACT engine is the steady-state pacer (~18.4us/pair of exp).
Each pair iteration interleaves at kc granularity: scores(c+1) groups are
round-robined with PV(c) partial accumulations and split qk(c+2) halves so
ACT receives a fresh score tile every ~2.3us and never starves at pair
boundaries.  PSUM: st2 (2x2 banks) rotates score tiles (also proj f_ps);
p2 (2x2 banks) holds ov pairs / qk / v transients.

Startup: single-shot DMAs split across both HWDGE queues (sync: xT, bq, bk,
Wq/Wk col-blocks 1:6; scalar: bv, bp, Wq/Wk col-block 0, Wv, Wp) so QT/KT
chunk 0 starts as soon as xT lands; dummy PE warmup matmuls flip the HAM
clock-gate to 8/8 before real compute arrives.
"""

import numpy as np

B, S, DIM, H = 8, 1024, 768, 12
HD = DIM // H          # 64
SCALE = HD ** -0.5
N_CORES = 8
KC = DIM // 128        # 6 d-chunks
SC = S // 128          # 8 seq-chunks
ENABLE_PINS = False

_CACHE = {}


def _build():
    import concourse.mybir as mybir
    import concourse.tile as tile
    from concourse import bacc

    f32 = mybir.dt.float32
    bf16 = mybir.dt.bfloat16
    EXP = mybir.ActivationFunctionType.Exp

    nc = bacc.Bacc()

    xT_ext = nc.declare_dram_parameter("xT", [DIM, S], bf16, isOutput=False)
    Wq_ext = nc.declare_dram_parameter("Wq", [DIM, DIM], bf16, isOutput=False)
    bq_ext = nc.declare_dram_parameter("bq", [DIM], f32, isOutput=False)
    Wk_ext = nc.declare_dram_parameter("Wk", [DIM, DIM], bf16, isOutput=False)
    bk_ext = nc.declare_dram_parameter("bk", [DIM], f32, isOutput=False)
    Wv_ext = nc.declare_dram_parameter("Wv", [DIM, DIM], bf16, isOutput=False)
    bv_ext = nc.declare_dram_parameter("bv", [DIM], bf16, isOutput=False)
    Wp_ext = nc.declare_dram_parameter("Wp", [DIM, DIM], bf16, isOutput=False)
    bp_ext = nc.declare_dram_parameter("bp", [DIM], bf16, isOutput=False)
    out_ext = nc.declare_dram_parameter("out", [S, DIM], bf16, isOutput=True)

    HALVES = ((0, 512), (512, 1024))
    VHALVES = ((0, 512), (512, DIM))

    with tile.TileContext(nc) as tc:
        with tc.tile_pool(name="persist", bufs=1) as sb, \
             tc.tile_pool(name="ps", bufs=1, space="PSUM") as ps:

            # p2: single-slot transient accumulator (2 banks) — all users are
            # strictly sequential (warmup, bias, qk, v, one ov at a time).
            # st2: 3-slot rotation (6 banks) for score tiles — the extra slot
            # gives the ACT exp stream a deeper backlog so scheduler slop no
            # longer starves it; norm's bc_ps borrows a slot transiently.
            def p2(name, shape=(128, S), dtype=f32):
                return ps.tile(list(shape), dtype, tag="p2", bufs=1, name=name)

            def st2(name):
                return ps.tile([128, S], f32, tag="st2", bufs=3, name=name)

            # ---- constants ----
            ones2d = sb.tile([128, 128], bf16)
            nc.vector.memset(ones2d, 1.0)
            rs_z = sb.tile([128, DIM], bf16)
            nc.vector.memset(rs_z, 0.0)
            rs_zs = sb.tile([128, S], bf16)
            nc.vector.memset(rs_zs, 0.0)

            # PE warmup: dummy matmuls keep the array busy from the end of the
            # framework preamble so the HAM clock-gate flips to 8/8 (~3.4us)
            # and stays there until the input DMAs land.
            warm_ps = p2("warm", (128, 512))
            for w in range(145):
                nc.tensor.matmul(
                    warm_ps[:, (w % 4) * 128:(w % 4) * 128 + 128],
                    ones2d, ones2d, start=True, stop=True)

            # ---- input DMAs: single-shot per tensor, split across queues.
            # Critical path first (x, Wq, Wk); Wv/Wp issues are deferred via
            # a dep on the first score matmul so their transfers do not steal
            # DMA bandwidth from the QT0/KT0 inputs.
            xsb = sb.tile([128, KC, S], bf16, name="xsb")
            nc.sync.dma_start(
                out=xsb, in_=xT_ext[:].rearrange("(c p) s -> p c s", p=128))
            Wq_sb = sb.tile([128, KC, DIM], bf16, name="Wq_sb")
            Wk_sb = sb.tile([128, KC, DIM], bf16, name="Wk_sb")
            # col-block 0 (needed by QT0/KT0) goes on the scalar queue so it
            # arrives with xT; the rest follows on sync.
            nc.sync.dma_start(
                out=Wq_sb[:, :, 128:DIM],
                in_=Wq_ext[:, 128:DIM].rearrange("(c p) n -> p c n", p=128))
            nc.sync.dma_start(
                out=Wk_sb[:, :, 128:DIM],
                in_=Wk_ext[:, 128:DIM].rearrange("(c p) n -> p c n", p=128))
            bq_sb = sb.tile([128, KC], f32)
            nc.sync.dma_start(out=bq_sb, in_=bq_ext[:].rearrange("(c p) -> p c", p=128))
            bk_sb = sb.tile([128, KC], f32)
            nc.sync.dma_start(out=bk_sb, in_=bk_ext[:].rearrange("(c p) -> p c", p=128))

            nc.scalar.dma_start(
                out=Wq_sb[:, :, 0:128],
                in_=Wq_ext[:, 0:128].rearrange("(c p) n -> p c n", p=128))
            nc.scalar.dma_start(
                out=Wk_sb[:, :, 0:128],
                in_=Wk_ext[:, 0:128].rearrange("(c p) n -> p c n", p=128))
            bv_row = sb.tile([1, DIM], bf16)
            nc.scalar.dma_start(out=bv_row, in_=bv_ext[:].rearrange("(a d) -> a d", a=1))
            bp_row = sb.tile([1, DIM], bf16)
            nc.scalar.dma_start(out=bp_row, in_=bp_ext[:].rearrange("(a d) -> a d", a=1))
            Wv_sb = sb.tile([128, KC, DIM], bf16, name="Wv_sb")
            wv_dma = nc.scalar.dma_start(
                out=Wv_sb, in_=Wv_ext[:].rearrange("(c p) n -> p c n", p=128))
            Wp_sb = sb.tile([128, KC, DIM], bf16, name="Wp_sb")
            wp_dma = nc.scalar.dma_start(
                out=Wp_sb, in_=Wp_ext[:].rearrange("(c p) n -> p c n", p=128))

            xT = [xsb[:, c, :] for c in range(KC)]

            # broadcast bv/bp across 128 partitions: full-util K=128 matmul
            # against the zero-padded carrier (row 0 = bias, rows 1:128 = 0).
            bv_bc = sb.tile([128, DIM], f32)
            bp_bc = sb.tile([128, DIM], f32)
            for row, bc in ((bv_row, bv_bc), (bp_row, bp_bc)):
                nc.vector.tensor_copy(rs_z[0:1, 0:DIM], row[0:1, :])
                bc_ps = p2("bias_ps")
                for n0, n1 in VHALVES:
                    nc.tensor.matmul(bc_ps[:, n0:n1], ones2d,
                                     rs_z[:, n0:n1], start=True, stop=True)
                nc.scalar.copy(bc, bc_ps[:, 0:DIM])

            # persistent activation storage; V12 col 64 is the ones column
            # that makes PV also emit the softmax rowsum (out row 64).
            QT = [sb.tile([128, S], bf16, name=f"QT{c}") for c in range(KC)]
            KT = [sb.tile([128, S], bf16, name=f"KT{c}") for c in range(KC)]
            V12 = [sb.tile([128, H, HD + 1], bf16, name=f"V12_{s8}") for s8 in range(SC)]
            for s8 in range(SC):
                nc.vector.memset(V12[s8][:, :, HD:HD + 1], 1.0)

            # Ordering pin: anchor filler blocks behind the most recent score
            # group so the greedy scheduler cannot hoist them ahead of it and
            # starve the ACT engine (the steady-state pacer).
            anchor = [None]

            def pin(inst):
                if ENABLE_PINS and anchor[0] is not None:
                    tile.add_dep_helper(inst.ins, anchor[0].ins, sync=False,
                                        reason="keep ACT fed")

            # QT / KT chunk m, split in two emission halves (k 0:3 / 3:6) so
            # the pair loop can interleave them between score groups.
            def qk_half(W_sb, bias_sb, dst, m, q_ps, first, pinned=False):
                ks = range(0, 3) if first else range(3, KC)
                for ki, k in enumerate(ks):
                    for ni, (n0, n1) in enumerate(HALVES):
                        mm = nc.tensor.matmul(
                            q_ps[:, n0:n1],
                            W_sb[:, k, m * 128:(m + 1) * 128],
                            xT[k][:, n0:n1],
                            start=(k == 0), stop=(k == KC - 1))
                        if pinned and ki == 0 and ni == 0:
                            pin(mm)
                if not first:
                    nc.vector.tensor_scalar_add(dst[m], q_ps, bias_sb[:, m:m + 1])

            def qk_chunk(W_sb, bias_sb, dst, m):
                q_ps = p2("q_ps")
                qk_half(W_sb, bias_sb, dst, m, q_ps, True)
                qk_half(W_sb, bias_sb, dst, m, q_ps, False)

            # V natural layout: out[seq, d] = x @ Wv; +bv broadcast; ->bf16.
            def emit_v_chunk(s8):
                v_ps = p2("v_ps")
                for k in range(KC):
                    for n0, n1 in VHALVES:
                        nc.tensor.matmul(
                            v_ps[:, n0:n1],
                            xT[k][:, s8 * 128:(s8 + 1) * 128],
                            Wv_sb[:, k, n0:n1],
                            start=(k == 0), stop=(k == KC - 1))
                nc.vector.tensor_add(
                    V12[s8][:, :, 0:HD],
                    v_ps[:, 0:DIM].rearrange("p (h d) -> p h d", h=H),
                    bv_bc[:].rearrange("p (h d) -> p h d", h=H))

            qk_chunk(Wq_sb, bq_sb, QT, 0)
            qk_chunk(Wk_sb, bk_sb, KT, 0)

            # ---- attention + output projection ----
            with tc.tile_pool(name="pb", bufs=1) as pb:
                OT = [pb.tile([128, S], bf16, name=f"OT{c}") for c in range(KC)]

                # Schraudolph fast-exp on DVE for the offloaded k-chunks:
                # exp(SCALE*s) = bitcast_f32(int32(EA*s + EB)).  Rebalances
                # the exp stream (the pacer) from ACT onto the idle DVE at
                # ~1.7% rms approximation error on those chunks.
                # Single DVE op: t = (EA*s + EB) in fp32, convert to int16
                # (round) and land the bits directly in the bf16 pt tile --
                # int16 steps are exactly bf16 ulps of 2^(scale*s/ln2).
                EA = float((1 << 23) * SCALE / np.log(2.0) / 65536.0)
                EB = float((127 * (1 << 23) - 366393) / 65536.0)
                OFF_KCS = ()
                i16 = mybir.dt.int16
                MULT, ADD = mybir.AluOpType.mult, mybir.AluOpType.add

                def dve_exp(dst, st):
                    nc.vector.tensor_scalar(
                        out=dst.bitcast(i16), in0=st, scalar1=EA, scalar2=EB,
                        op0=MULT, op1=ADD)

                def sc_group(c, kc, pt):
                    # scores + exp for (pair c, k-chunk kc): row-packed
                    # matmuls on array halves run concurrently.
                    st_e = st2("st_e")
                    st_o = st2("st_o")
                    for n0, n1 in HALVES:
                        nc.tensor.matmul(
                            st_e[:, n0:n1],
                            KT[c][0:HD, kc * 128:(kc + 1) * 128],
                            QT[c][0:HD, n0:n1],
                            start=True, stop=True)
                        mm = nc.tensor.matmul(
                            st_o[:, n0:n1],
                            KT[c][HD:128, kc * 128:(kc + 1) * 128],
                            QT[c][HD:128, n0:n1],
                            start=True, stop=True)
                    anchor[0] = mm
                    p_e = pb.tile([128, S], bf16, tag=f"pt{kc}e", bufs=2, name=f"pt{kc}e")
                    p_o = pb.tile([128, S], bf16, tag=f"pt{kc}o", bufs=2, name=f"pt{kc}o")
                    if kc in OFF_KCS:
                        dve_exp(p_e, st_e)
                        dve_exp(p_o, st_o)
                    else:
                        nc.scalar.activation(p_e, st_e, EXP, scale=SCALE)
                        nc.scalar.activation(p_o, st_o, EXP, scale=SCALE)
                    pt[0].append(p_e)
                    pt[1].append(p_o)

                def pv_part(c, half, ptl, ov, kcs, pinned=True):
                    for j, kc in enumerate(kcs):
                        for ni, (n0, n1) in enumerate(HALVES):
                            mm = nc.tensor.matmul(
                                ov[0:HD + 1, n0:n1],
                                V12[kc][:, 2 * c + half, :],
                                ptl[kc][:, n0:n1],
                                start=(kc == 0), stop=(kc == SC - 1))
                            if pinned and j == 0 and ni == 0:
                                pin(mm)

                def norm(c, half, ov):
                    # rowsum (row 64 of ov) -> zero-padded carrier ->
                    # broadcast via full-util K=128 matmul -> 1/x -> normalize
                    nc.vector.tensor_copy(rs_zs[0:1, :], ov[HD:HD + 1, :])
                    bc_ps = st2("bc_ps")
                    for ni, (n0, n1) in enumerate(HALVES):
                        mm = nc.tensor.matmul(bc_ps[:, n0:n1], ones2d,
                                              rs_zs[:, n0:n1], start=True, stop=True)
                        if ni == 0:
                            pin(mm)
                    rbc = pb.tile([HD, S], f32, tag="rbc", bufs=2, name="rbc")
                    nc.vector.reciprocal_approx_fast(rbc, bc_ps[0:HD, :])
                    base = half * HD
                    nc.vector.tensor_mul(OT[c][base:base + HD, :], ov[0:HD, :], rbc)

                # pair 0 fill: score groups first (the exp stream must start
                # flowing), then qk(1) (pair 1's scores gate on it), then the
                # V chunks — unpinned, so the scheduler drops them into the
                # ACT-paced PE stalls.
                pts = ([], [])
                for kc in range(SC):
                    sc_group(0, kc, pts)
                    if kc == 0:
                        # release the deferred Wv/Wp transfers once the
                        # critical-path inputs have landed
                        tile.add_dep_helper(wv_dma.ins, anchor[0].ins,
                                            reason="defer Wv transfer")
                        tile.add_dep_helper(wp_dma.ins, anchor[0].ins,
                                            reason="defer Wp transfer")
                qk_chunk(Wq_sb, bq_sb, QT, 1)
                qk_chunk(Wk_sb, bk_sb, KT, 1)
                for kc in range(SC):
                    emit_v_chunk(kc)

                # steady state: scores(c+1) round-robined with PV(c) partials
                # and qk(c+2) halves, ACT-paced.
                for c in range(KC):
                    last = c + 1 >= KC
                    ptn = ([], [])

                    def sc(kc):
                        if not last:
                            sc_group(c + 1, kc, ptn)

                    sc(0)
                    sc(1)
                    ov0 = p2("ov0")
                    # first filler of the iteration is unpinned: it is the
                    # boundary slack-filler while ACT drains the prior pair
                    pv_part(c, 0, pts[0], ov0, range(0, 4), pinned=False)
                    sc(2)
                    pv_part(c, 0, pts[0], ov0, range(4, SC))
                    sc(3)
                    norm(c, 0, ov0)
                    # single-slot p2: ov1 reuses ov0's slot once norm(c,0)'s
                    # multiply has drained
                    ov1 = p2("ov1")
                    pv_part(c, 1, pts[1], ov1, range(0, 4))
                    sc(4)
                    pv_part(c, 1, pts[1], ov1, range(4, SC))
                    sc(5)
                    norm(c, 1, ov1)
                    if c + 2 < KC:
                        q_psq = p2("q_ps")
                        qk_half(Wq_sb, bq_sb, QT, c + 2, q_psq, True, pinned=True)
                        sc(6)
                        qk_half(Wq_sb, bq_sb, QT, c + 2, q_psq, False, pinned=True)
                        sc(7)
                        q_psk = p2("q_ps")
                        qk_half(Wk_sb, bk_sb, KT, c + 2, q_psk, True, pinned=True)
                        qk_half(Wk_sb, bk_sb, KT, c + 2, q_psk, False)
                    else:
                        sc(6)
                        sc(7)
                    pts = ptn

                # out = O_cat @ Wp + bp  (bf16 store, host casts to f32).
                # Split each chunk's contraction at k=5 so the k=0..4 partial
                # (10 MMs) can overlap the last pair's PV/norm drain; only
                # the k=5 matmuls wait on the final heads.
                def proj_part(s8, f_ps, ks):
                    for k in ks:
                        for n0, n1 in VHALVES:
                            nc.tensor.matmul(
                                f_ps[:, n0:n1],
                                OT[k][:, s8 * 128:(s8 + 1) * 128],
                                Wp_sb[:, k, n0:n1],
                                start=(k == 0), stop=(k == KC - 1))

                def proj_fin(s8, f_ps):
                    proj_part(s8, f_ps, range(KC - 1, KC))
                    fin = pb.tile([128, DIM], bf16, tag="fin", bufs=2, name="fin")
                    nc.vector.tensor_add(fin, f_ps[:, 0:DIM], bp_bc)
                    eng = nc.sync if s8 % 2 == 0 else nc.scalar
                    eng.dma_start(out=out_ext[s8 * 128:(s8 + 1) * 128, :], in_=fin)

                for s8 in range(SC):
                    f_ps = st2("f_ps")
                    proj_part(s8, f_ps, range(KC - 1))
                    proj_fin(s8, f_ps)

    nc.compile()
    return nc


def get_nc():
    if "nc" not in _CACHE:
        _CACHE["nc"] = _build()
    return _CACHE["nc"]


def make_in_maps(x, Wq, bq, Wk, bk, Wv, bv, Wp, bp):
    import ml_dtypes
    bfl = ml_dtypes.bfloat16
    shared = {
        "Wq": np.ascontiguousarray(np.asarray(Wq, np.float32).astype(bfl)),
        "bq": np.ascontiguousarray(np.asarray(bq, np.float32)),
        "Wk": np.ascontiguousarray(np.asarray(Wk, np.float32).astype(bfl)),
        "bk": np.ascontiguousarray(np.asarray(bk, np.float32)),
        "Wv": np.ascontiguousarray(np.asarray(Wv, np.float32).astype(bfl)),
        "bv": np.ascontiguousarray(np.asarray(bv, np.float32).astype(bfl)),
        "Wp": np.ascontiguousarray(np.asarray(Wp, np.float32).astype(bfl)),
        "bp": np.ascontiguousarray(np.asarray(bp, np.float32).astype(bfl)),
    }
    xb = np.asarray(x, np.float32).astype(bfl)
    return [{"xT": np.ascontiguousarray(xb[b].T), **shared} for b in range(N_CORES)]


def kernel(x, Wq, bq, Wk, bk, Wv, bv, Wp, bp):
    from concourse.bass_utils import run_bass_kernel_spmd

    nc = get_nc()
    in_maps = make_in_maps(x, Wq, bq, Wk, bk, Wv, bv, Wp, bp)
    res = run_bass_kernel_spmd(nc, in_maps, core_ids=list(range(N_CORES)))
    return np.stack(
        [res.results[i]["out"].astype(np.float32) for i in range(N_CORES)], axis=0)


# revision 42
# speedup vs baseline: 1.1617x; 1.1473x over previous
"""Multi-head attention block (B=8, S=1024, D=768, H=12) on 8 TRN2 NeuronCores.

Data-parallel: one batch element per core (attention is independent per batch).
Per-core pipeline (bf16 matmuls, fp32 PSUM accumulation):

  xT [D,S] uploaded pre-transposed from host (bf16), single DMA
  QT = Wq^T xT (+bq), KT = Wk^T xT (+bk)      [D,S]  (head pairs per 128-chunk)
  V  = x Wv (+bv)                             [S,D]  stored as [128,12,65] with
                                                      a ones column per head
  per head pair (2c,2c+1):
      S^T pair = K Q^T row-packed on PE array halves (two [128,1024] PSUM tiles)
      exp(scale*S^T) -> PT pair [Sk, Sq] bf16 per head
      per head: O'^T = [V_h,1]^T PT (accum Sk) -> rows 0:64 = O^T, row 64 =
                rowsum; OT_h = O'^T[0:64] * (1 / bcast(rowsum))
  out = O_cat @ Wp (+bp)                      [S,D]  (bf16 output, host-cast f32)

Schedule: the ACT engine is the steady-state pacer (~18.4us/pair of exp).
Each pair iteration interleaves at kc granularity: scores(c+1) groups are
round-robined with PV(c) partial accumulations and split qk(c+2) halves so
ACT receives a fresh score tile every ~2.3us and never starves at pair
boundaries.  PSUM: st2 (2x2 banks) rotates score tiles (also proj f_ps);
p2 (2x2 banks) holds ov pairs / qk / v transients.

Startup: single-shot DMAs split across both HWDGE queues (sync: xT, bq, bk,
Wq/Wk col-blocks 1:6; scalar: bv, bp, Wq/Wk col-block 0, Wv, Wp) so QT/KT
chunk 0 starts as soon as xT lands; dummy PE warmup matmuls flip the HAM
clock-gate to 8/8 before real compute arrives.
"""

import numpy as np

B, S, DIM, H = 8, 1024, 768, 12
HD = DIM // H          # 64
SCALE = HD ** -0.5
N_CORES = 8
KC = DIM // 128        # 6 d-chunks
SC = S // 128          # 8 seq-chunks
ENABLE_PINS = False

_CACHE = {}


def _build():
    import concourse.mybir as mybir
    import concourse.tile as tile
    from concourse import bacc

    f32 = mybir.dt.float32
    bf16 = mybir.dt.bfloat16
    EXP = mybir.ActivationFunctionType.Exp

    nc = bacc.Bacc()

    xT_ext = nc.declare_dram_parameter("xT", [DIM, S], bf16, isOutput=False)
    Wq_ext = nc.declare_dram_parameter("Wq", [DIM, DIM], bf16, isOutput=False)
    bq_ext = nc.declare_dram_parameter("bq", [DIM], f32, isOutput=False)
    Wk_ext = nc.declare_dram_parameter("Wk", [DIM, DIM], bf16, isOutput=False)
    bk_ext = nc.declare_dram_parameter("bk", [DIM], f32, isOutput=False)
    Wv_ext = nc.declare_dram_parameter("Wv", [DIM, DIM], bf16, isOutput=False)
    bv_ext = nc.declare_dram_parameter("bv", [DIM], bf16, isOutput=False)
    Wp_ext = nc.declare_dram_parameter("Wp", [DIM, DIM], bf16, isOutput=False)
    bp_ext = nc.declare_dram_parameter("bp", [DIM], bf16, isOutput=False)
    out_ext = nc.declare_dram_parameter("out", [S, DIM], bf16, isOutput=True)

    HALVES = ((0, 512), (512, 1024))
    VHALVES = ((0, 512), (512, DIM))

    with tile.TileContext(nc) as tc:
        with tc.tile_pool(name="persist", bufs=1) as sb, \
             tc.tile_pool(name="ps", bufs=1, space="PSUM") as ps:

            # p2: single-slot transient accumulator (2 banks) — all users are
            # strictly sequential (warmup, bias, qk, v, one ov at a time).
            # st2: 3-slot rotation (6 banks) for score tiles — the extra slot
            # gives the ACT exp stream a deeper backlog so scheduler slop no
            # longer starves it; norm's bc_ps borrows a slot transiently.
            def p2(name, shape=(128, S), dtype=f32):
                return ps.tile(list(shape), dtype, tag="p2", bufs=1, name=name)

            def st2(name):
                return ps.tile([128, S], f32, tag="st2", bufs=3, name=name)

            # ---- constants ----
            ones2d = sb.tile([128, 128], bf16)
            nc.vector.memset(ones2d, 1.0)
            rs_z = sb.tile([128, DIM], bf16)
            nc.vector.memset(rs_z, 0.0)
            rs_zs = sb.tile([128, S], bf16)
            nc.vector.memset(rs_zs, 0.0)

            # PE warmup: dummy matmuls keep the array busy from the end of the
            # framework preamble so the HAM clock-gate flips to 8/8 (~3.4us)
            # and stays there until the input DMAs land.
            warm_ps = p2("warm", (128, 512))
            for w in range(145):
                nc.tensor.matmul(
                    warm_ps[:, (w % 4) * 128:(w % 4) * 128 + 128],
                    ones2d, ones2d, start=True, stop=True)

            # ---- input DMAs: single-shot per tensor, split across queues.
            # Critical path first (x, Wq, Wk); Wv/Wp issues are deferred via
            # a dep on the first score matmul so their transfers do not steal
            # DMA bandwidth from the QT0/KT0 inputs.
            xsb = sb.tile([128, KC, S], bf16, name="xsb")
            nc.sync.dma_start(
                out=xsb, in_=xT_ext[:].rearrange("(c p) s -> p c s", p=128))
            Wq_sb = sb.tile([128, KC, DIM], bf16, name="Wq_sb")
            Wk_sb = sb.tile([128, KC, DIM], bf16, name="Wk_sb")
            # col-block 0 (needed by QT0/KT0) goes on the scalar queue so it
            # arrives with xT; the rest follows on sync.
            nc.sync.dma_start(
                out=Wq_sb[:, :, 128:DIM],
                in_=Wq_ext[:, 128:DIM].rearrange("(c p) n -> p c n", p=128))
            nc.sync.dma_start(
                out=Wk_sb[:, :, 128:DIM],
                in_=Wk_ext[:, 128:DIM].rearrange("(c p) n -> p c n", p=128))
            bq_sb = sb.tile([128, KC], f32)
            nc.sync.dma_start(out=bq_sb, in_=bq_ext[:].rearrange("(c p) -> p c", p=128))
            bk_sb = sb.tile([128, KC], f32)
            nc.sync.dma_start(out=bk_sb, in_=bk_ext[:].rearrange("(c p) -> p c", p=128))

            nc.scalar.dma_start(
                out=Wq_sb[:, :, 0:128],
                in_=Wq_ext[:, 0:128].rearrange("(c p) n -> p c n", p=128))
            nc.scalar.dma_start(
                out=Wk_sb[:, :, 0:128],
                in_=Wk_ext[:, 0:128].rearrange("(c p) n -> p c n", p=128))
            bv_row = sb.tile([1, DIM], bf16)
            nc.scalar.dma_start(out=bv_row, in_=bv_ext[:].rearrange("(a d) -> a d", a=1))
            bp_row = sb.tile([1, DIM], bf16)
            nc.scalar.dma_start(out=bp_row, in_=bp_ext[:].rearrange("(a d) -> a d", a=1))
            Wv_sb = sb.tile([128, KC, DIM], bf16, name="Wv_sb")
            wv_dma = nc.scalar.dma_start(
                out=Wv_sb, in_=Wv_ext[:].rearrange("(c p) n -> p c n", p=128))
            Wp_sb = sb.tile([128, KC, DIM], bf16, name="Wp_sb")
            wp_dma = nc.scalar.dma_start(
                out=Wp_sb, in_=Wp_ext[:].rearrange("(c p) n -> p c n", p=128))

            xT = [xsb[:, c, :] for c in range(KC)]

            # broadcast bv/bp across 128 partitions: full-util K=128 matmul
            # against the zero-padded carrier (row 0 = bias, rows 1:128 = 0).
            bv_bc = sb.tile([128, DIM], f32)
            bp_bc = sb.tile([128, DIM], f32)
            for row, bc in ((bv_row, bv_bc), (bp_row, bp_bc)):
                nc.vector.tensor_copy(rs_z[0:1, 0:DIM], row[0:1, :])
                bc_ps = p2("bias_ps")
                for n0, n1 in VHALVES:
                    nc.tensor.matmul(bc_ps[:, n0:n1], ones2d,
                                     rs_z[:, n0:n1], start=True, stop=True)
                nc.scalar.copy(bc, bc_ps[:, 0:DIM])

            # persistent activation storage; V12 col 64 is the ones column
            # that makes PV also emit the softmax rowsum (out row 64).
            QT = [sb.tile([128, S], bf16, name=f"QT{c}") for c in range(KC)]
            KT = [sb.tile([128, S], bf16, name=f"KT{c}") for c in range(KC)]
            V12 = [sb.tile([128, H, HD + 1], bf16, name=f"V12_{s8}") for s8 in range(SC)]
            for s8 in range(SC):
                nc.vector.memset(V12[s8][:, :, HD:HD + 1], 1.0)

            # Ordering pin: anchor filler blocks behind the most recent score
            # group so the greedy scheduler cannot hoist them ahead of it and
            # starve the ACT engine (the steady-state pacer).
            anchor = [None]

            def pin(inst):
                if ENABLE_PINS and anchor[0] is not None:
                    tile.add_dep_helper(inst.ins, anchor[0].ins, sync=False,
                                        reason="keep ACT fed")

            # QT / KT chunk m, split in two emission halves (k 0:3 / 3:6) so
            # the pair loop can interleave them between score groups.
            def qk_half(W_sb, bias_sb, dst, m, q_ps, first, pinned=False):
                ks = range(0, 3) if first else range(3, KC)
                for ki, k in enumerate(ks):
                    for ni, (n0, n1) in enumerate(HALVES):
                        mm = nc.tensor.matmul(
                            q_ps[:, n0:n1],
                            W_sb[:, k, m * 128:(m + 1) * 128],
                            xT[k][:, n0:n1],
                            start=(k == 0), stop=(k == KC - 1))
                        if pinned and ki == 0 and ni == 0:
                            pin(mm)
                if not first:
                    nc.vector.tensor_scalar_add(dst[m], q_ps, bias_sb[:, m:m + 1])

            def qk_chunk(W_sb, bias_sb, dst, m):
                q_ps = p2("q_ps")
                qk_half(W_sb, bias_sb, dst, m, q_ps, True)
                qk_half(W_sb, bias_sb, dst, m, q_ps, False)

            # V natural layout: out[seq, d] = x @ Wv; +bv broadcast; ->bf16.
            def emit_v_chunk(s8):
                v_ps = p2("v_ps")
                for k in range(KC):
                    for n0, n1 in VHALVES:
                        nc.tensor.matmul(
                            v_ps[:, n0:n1],
                            xT[k][:, s8 * 128:(s8 + 1) * 128],
                            Wv_sb[:, k, n0:n1],
                            start=(k == 0), stop=(k == KC - 1))
                nc.vector.tensor_add(
                    V12[s8][:, :, 0:HD],
                    v_ps[:, 0:DIM].rearrange("p (h d) -> p h d", h=H),
                    bv_bc[:].rearrange("p (h d) -> p h d", h=H))

            qk_chunk(Wq_sb, bq_sb, QT, 0)
            qk_chunk(Wk_sb, bk_sb, KT, 0)

            # ---- attention + output projection ----
            with tc.tile_pool(name="pb", bufs=1) as pb:
                OT = [pb.tile([128, S], bf16, name=f"OT{c}") for c in range(KC)]

                # Schraudolph fast-exp on DVE for the offloaded k-chunks:
                # exp(SCALE*s) = bitcast_f32(int32(EA*s + EB)).  Rebalances
                # the exp stream (the pacer) from ACT onto the idle DVE at
                # ~1.7% rms approximation error on those chunks.
                # Single DVE op: t = (EA*s + EB) in fp32, convert to int16
                # (round) and land the bits directly in the bf16 pt tile --
                # int16 steps are exactly bf16 ulps of 2^(scale*s/ln2).
                EA = float((1 << 23) * SCALE / np.log(2.0) / 65536.0)
                EB = float((127 * (1 << 23) - 366393) / 65536.0)
                OFF_KCS = ()
                i16 = mybir.dt.int16
                MULT, ADD = mybir.AluOpType.mult, mybir.AluOpType.add

                def dve_exp(dst, st):
                    nc.vector.tensor_scalar(
                        out=dst.bitcast(i16), in0=st, scalar1=EA, scalar2=EB,
                        op0=MULT, op1=ADD)

                def sc_group(c, kc, pt):
                    # scores + exp for (pair c, k-chunk kc): row-packed
                    # matmuls on array halves run concurrently.
                    st_e = st2("st_e")
                    st_o = st2("st_o")
                    for n0, n1 in HALVES:
                        nc.tensor.matmul(
                            st_e[:, n0:n1],
                            KT[c][0:HD, kc * 128:(kc + 1) * 128],
                            QT[c][0:HD, n0:n1],
                            start=True, stop=True)
                        mm = nc.tensor.matmul(
                            st_o[:, n0:n1],
                            KT[c][HD:128, kc * 128:(kc + 1) * 128],
                            QT[c][HD:128, n0:n1],
                            start=True, stop=True)
                    anchor[0] = mm
                    p_e = pb.tile([128, S], bf16, tag=f"pt{kc}e", bufs=2, name=f"pt{kc}e")
                    p_o = pb.tile([128, S], bf16, tag=f"pt{kc}o", bufs=2, name=f"pt{kc}o")
                    if kc in OFF_KCS:
                        dve_exp(p_e, st_e)
                        dve_exp(p_o, st_o)
                    else:
                        nc.scalar.activation(p_e, st_e, EXP, scale=SCALE)
                        nc.scalar.activation(p_o, st_o, EXP, scale=SCALE)
                    pt[0].append(p_e)
                    pt[1].append(p_o)

                def pv_part(c, half, ptl, ov, kcs, pinned=True):
                    for j, kc in enumerate(kcs):
                        for ni, (n0, n1) in enumerate(HALVES):
                            mm = nc.tensor.matmul(
                                ov[0:HD + 1, n0:n1],
                                V12[kc][:, 2 * c + half, :],
                                ptl[kc][:, n0:n1],
                                start=(kc == 0), stop=(kc == SC - 1))
                            if pinned and j == 0 and ni == 0:
                                pin(mm)

                def norm(c, half, ov):
                    # rowsum (row 64 of ov) -> zero-padded carrier ->
                    # broadcast via full-util K=128 matmul -> 1/x -> normalize
                    nc.vector.tensor_copy(rs_zs[0:1, :], ov[HD:HD + 1, :])
                    bc_ps = st2("bc_ps")
                    for ni, (n0, n1) in enumerate(HALVES):
                        mm = nc.tensor.matmul(bc_ps[:, n0:n1], ones2d,
                                              rs_zs[:, n0:n1], start=True, stop=True)
                        if ni == 0:
                            pin(mm)
                    rbc = pb.tile([HD, S], f32, tag="rbc", bufs=2, name="rbc")
                    nc.vector.reciprocal_approx_fast(rbc, bc_ps[0:HD, :])
                    base = half * HD
                    nc.vector.tensor_mul(OT[c][base:base + HD, :], ov[0:HD, :], rbc)

                # pair 0 fill: score groups first (the exp stream must start
                # flowing), then qk(1) (pair 1's scores gate on it), then the
                # V chunks — unpinned, so the scheduler drops them into the
                # ACT-paced PE stalls.
                pts = ([], [])
                for kc in range(5):
                    sc_group(0, kc, pts)
                    if kc == 0:
                        # release the deferred Wv/Wp transfers once the
                        # critical-path inputs have landed
                        tile.add_dep_helper(wv_dma.ins, anchor[0].ins,
                                            reason="defer Wv transfer")
                        tile.add_dep_helper(wp_dma.ins, anchor[0].ins,
                                            reason="defer Wp transfer")
                # qk(1) halves ride the ACT-paced PE stalls between the late
                # pair-0 score groups (Wq/Wk have landed by this point), so
                # pair-1 scores can start right after pair-0's last exp.
                q_psq = p2("q_ps")
                qk_half(Wq_sb, bq_sb, QT, 1, q_psq, True)
                sc_group(0, 5, pts)
                qk_half(Wq_sb, bq_sb, QT, 1, q_psq, False)
                sc_group(0, 6, pts)
                q_psk = p2("q_ps")
                qk_half(Wk_sb, bk_sb, KT, 1, q_psk, True)
                sc_group(0, 7, pts)
                qk_half(Wk_sb, bk_sb, KT, 1, q_psk, False)
                for kc in range(SC):
                    emit_v_chunk(kc)

                # steady state: scores(c+1) round-robined with PV(c) partials
                # and qk(c+2) halves, ACT-paced.
                for c in range(KC):
                    last = c + 1 >= KC
                    ptn = ([], [])

                    def sc(kc):
                        if not last:
                            sc_group(c + 1, kc, ptn)

                    sc(0)
                    sc(1)
                    ov0 = p2("ov0")
                    # first filler of the iteration is unpinned: it is the
                    # boundary slack-filler while ACT drains the prior pair
                    pv_part(c, 0, pts[0], ov0, range(0, 4), pinned=False)
                    sc(2)
                    pv_part(c, 0, pts[0], ov0, range(4, SC))
                    sc(3)
                    norm(c, 0, ov0)
                    # single-slot p2: ov1 reuses ov0's slot once norm(c,0)'s
                    # multiply has drained
                    ov1 = p2("ov1")
                    pv_part(c, 1, pts[1], ov1, range(0, 4))
                    sc(4)
                    pv_part(c, 1, pts[1], ov1, range(4, SC))
                    sc(5)
                    norm(c, 1, ov1)
                    if c + 2 < KC:
                        q_psq = p2("q_ps")
                        qk_half(Wq_sb, bq_sb, QT, c + 2, q_psq, True, pinned=True)
                        sc(6)
                        qk_half(Wq_sb, bq_sb, QT, c + 2, q_psq, False, pinned=True)
                        sc(7)
                        q_psk = p2("q_ps")
                        qk_half(Wk_sb, bk_sb, KT, c + 2, q_psk, True, pinned=True)
                        qk_half(Wk_sb, bk_sb, KT, c + 2, q_psk, False)
                    else:
                        sc(6)
                        sc(7)
                    pts = ptn

                # out = O_cat @ Wp + bp  (bf16 store, host casts to f32).
                # Split each chunk's contraction at k=5 so the k=0..4 partial
                # (10 MMs) can overlap the last pair's PV/norm drain; only
                # the k=5 matmuls wait on the final heads.
                def proj_part(s8, f_ps, ks):
                    for k in ks:
                        for n0, n1 in VHALVES:
                            nc.tensor.matmul(
                                f_ps[:, n0:n1],
                                OT[k][:, s8 * 128:(s8 + 1) * 128],
                                Wp_sb[:, k, n0:n1],
                                start=(k == 0), stop=(k == KC - 1))

                def proj_fin(s8, f_ps):
                    proj_part(s8, f_ps, range(KC - 1, KC))
                    fin = pb.tile([128, DIM], bf16, tag="fin", bufs=2, name="fin")
                    nc.vector.tensor_add(fin, f_ps[:, 0:DIM], bp_bc)
                    eng = nc.sync if s8 % 2 == 0 else nc.scalar
                    eng.dma_start(out=out_ext[s8 * 128:(s8 + 1) * 128, :], in_=fin)

                for s8 in range(SC):
                    f_ps = st2("f_ps")
                    proj_part(s8, f_ps, range(KC - 1))
                    proj_fin(s8, f_ps)

    nc.compile()
    return nc


def get_nc():
    if "nc" not in _CACHE:
        _CACHE["nc"] = _build()
    return _CACHE["nc"]


def make_in_maps(x, Wq, bq, Wk, bk, Wv, bv, Wp, bp):
    import ml_dtypes
    bfl = ml_dtypes.bfloat16
    shared = {
        "Wq": np.ascontiguousarray(np.asarray(Wq, np.float32).astype(bfl)),
        "bq": np.ascontiguousarray(np.asarray(bq, np.float32)),
        "Wk": np.ascontiguousarray(np.asarray(Wk, np.float32).astype(bfl)),
        "bk": np.ascontiguousarray(np.asarray(bk, np.float32)),
        "Wv": np.ascontiguousarray(np.asarray(Wv, np.float32).astype(bfl)),
        "bv": np.ascontiguousarray(np.asarray(bv, np.float32).astype(bfl)),
        "Wp": np.ascontiguousarray(np.asarray(Wp, np.float32).astype(bfl)),
        "bp": np.ascontiguousarray(np.asarray(bp, np.float32).astype(bfl)),
    }
    xb = np.asarray(x, np.float32).astype(bfl)
    return [{"xT": np.ascontiguousarray(xb[b].T), **shared} for b in range(N_CORES)]


def kernel(x, Wq, bq, Wk, bk, Wv, bv, Wp, bp):
    from concourse.bass_utils import run_bass_kernel_spmd

    nc = get_nc()
    in_maps = make_in_maps(x, Wq, bq, Wk, bk, Wv, bv, Wp, bp)
    res = run_bass_kernel_spmd(nc, in_maps, core_ids=list(range(N_CORES)))
    return np.stack(
        [res.results[i]["out"].astype(np.float32) for i in range(N_CORES)], axis=0)
